# revision 11
# baseline (speedup 1.0000x reference)
"""AttentionBlock (GroupNorm32 + QKV 8-head attention + proj + residual) on 8 TRN2 NeuronCores.

Sharding: pure data-parallel over batch B=8 - one batch element per core.

v2 design (fp8 + restructured schedule). Per-core engine walls: ACT must run
64 softmax Exps of 1024 elems (~73us busy - the only engine with exp); the PE
matmul stream is cut below that wall with fp8 DoubleRow (contraction 256/pass)
for the qkv projections, v, and AV (q/k/QK and the x-residual stay bf16 for
accuracy; numerically validated ~5.7e-3 total vs 2e-2 budget):

  - front: input DMAs issued on BOTH hardware DGE queues (Sync + ACT);
    GN stats split DVE (4 sums + 2 sumsq via tensor_tensor_reduce) / ACT
    (2 squares); memsets (ones block, exp bias) on the idle GpSimd; a
    warm-matmul burst holds the HAM clock gate; first Exp at ~16us vs 28.
  - attention: blocks ordered n-OUTER ((p,0) for all p, then (p,1)) so all
    nh=0 attention outputs exist by block 3; proj out-tiles for nh=0 are
    fully contracted + evacuated + DMA'd DURING blocks 4-6 - half the output
    ships mid-attention. exp writes fp8 ew into a persistent 8-slot ring
    (one tensor, so DoubleRow AV can take a [128,2,512] sm-pair AP).
  - AV: stationary [v | ones] fp8 per (sm,h); DoubleRow contracts an
    sm-PAIR per matmul; psum rows 64:128 hold softmax row-sums for free.
  - fillers: kq/v units are naturally micro in fp8 (2 DoubleRow matmuls);
    pumped between sm's (with per-block deadlines) so neither the QK->exp
    chain nor the AV v-dependency ever stalls.
  - tail: only the nh=1 half remains: a3 evac, m2/m3 runs in the freed qk
    psum banks, finals, evacs, DMA issues split across Sync/ACT queues.
"""

import numpy as np
import ml_dtypes
from contextlib import ExitStack

import concourse.bass as bass
import concourse.tile as tile
from concourse import bacc, mybir
from concourse.bass_utils import run_bass_kernel_spmd

F32 = mybir.dt.float32
BF = mybir.dt.bfloat16
F8 = mybir.dt.float8e4
MULT = mybir.AluOpType.mult
ADD = mybir.AluOpType.add
AFT = mybir.ActivationFunctionType
AXX = mybir.AxisListType.X
DR = mybir.MatmulPerfMode.DoubleRow

C, T, H, CH = 512, 1024, 8, 64
NJ = C // 128          # 4 c-tiles
NTM = T // 128         # 8 t-tiles
NPJ = 8                # proj contraction chunks: [pw | I]
EXP_SCALE = float(CH) ** -0.5  # folded (q*s)*(k*s) scale, s = ch**-0.25
EXP_OFF = 4.0                  # exp(l - 4): keeps fp8 ew in e4m3 range
GN_N = 16 * T          # elements per group
WS = 8.0               # host weight scale (w8 = fp8(w*8))
XS = 2.0               # xn scale (xn8 = fp8(2*xn), folded into gn affine)
KQ_DS = 1.0 / (WS * XS)  # psum -> true k/q/v

BF_NP = ml_dtypes.bfloat16
F8_NP = ml_dtypes.float8_e4m3


def build_graph(enable_asserts: bool = False, use_act_dma: bool = True,
                use_gp_memset: bool = True, use_dr: bool = True):
    nc = bacc.Bacc(
        "TRN2",
        target_bir_lowering=False,
        debug=False,
        enable_asserts=enable_asserts,
    )
    act_q = nc.scalar if use_act_dma else nc.sync
    gp_q = nc.gpsimd if use_gp_memset else nc.vector

    def dr_mm(out, lhsT3, rhs3, start, stop):
        # one fp8 DoubleRow matmul over a k-tile pair, or two plain matmuls
        if use_dr:
            nc.tensor.matmul(out, lhsT3, rhs3, start=start, stop=stop,
                             perf_mode=DR)
        else:
            nc.tensor.matmul(out, lhsT3[:, 0, :], rhs3[:, 0, :],
                             start=start, stop=False)
            nc.tensor.matmul(out, lhsT3[:, 1, :], rhs3[:, 1, :],
                             start=False, stop=stop)
    xbf_d = nc.dram_tensor("xbf", [C, T], BF, kind="ExternalInput").ap()
    wq_d = nc.dram_tensor("wq", [C, C], F8, kind="ExternalInput").ap()
    wk_d = nc.dram_tensor("wk", [C, C], F8, kind="ExternalInput").ap()
    wv_d = nc.dram_tensor("wv", [C, C], F8, kind="ExternalInput").ap()
    pw_d = nc.dram_tensor("pw", [2 * C, C], BF, kind="ExternalInput").ap()
    cp_d = nc.dram_tensor("cpack", [128, 28], F32, kind="ExternalInput").ap()
    gt8_d = nc.dram_tensor("gt8", [8, 128], F32, kind="ExternalInput").ap()
    out_d = nc.dram_tensor("out", [C, T], F32, kind="ExternalOutput").ap()

    with tile.TileContext(nc) as tc, ExitStack() as ctx:
        consts = ctx.enter_context(tc.tile_pool(name="consts", bufs=1))
        bigs = ctx.enter_context(tc.tile_pool(name="bigs", bufs=1))
        work = ctx.enter_context(tc.tile_pool(name="work", bufs=4))
        rinvp = ctx.enter_context(tc.tile_pool(name="rinvp", bufs=4))
        outp = ctx.enter_context(tc.tile_pool(name="outp", bufs=4))
        qk_ps = ctx.enter_context(tc.tile_pool(name="qk_ps", bufs=2, space="PSUM"))
        kv_ps = ctx.enter_context(tc.tile_pool(name="kv_ps", bufs=2, space="PSUM"))
        av_ps = ctx.enter_context(tc.tile_pool(name="av_ps", bufs=2, space="PSUM"))

        # ---- persistent sbuf tensors ----
        xbf = bigs.tile([128, NJ, T], BF)       # bf16 x (GN stats + residual)
        xn8 = bigs.tile([128, NJ, T], F8)       # 2*groupnormed x (fp8)
        q_sb = bigs.tile([128, NJ, T], BF)      # q rows (head-major)
        k_sb = bigs.tile([128, NJ, T], BF)      # k rows (head-major)
        vT2 = bigs.tile([128, NTM, H, 128], F8)  # [v | ones] per (tm, head)
        ew_ring = bigs.tile([128, NTM, 2, 512], F8)  # exp ring: (sm%8, u, t)
        a_sb = bigs.tile([128, NJ, T], BF)      # normalized attention output

        # ---- DMAs split across both HWDGE queues (Sync + ACT) ----
        nc.sync.dma_start(xbf[:, 0, :], xbf_d[0:128, :])
        nc.sync.dma_start(xbf[:, 2, :], xbf_d[256:384, :])
        cpk = consts.tile([128, 28], F32)
        nc.sync.dma_start(cpk[:], cp_d[:])
        gt8_sb = consts.tile([8, 128], F32)
        nc.sync.dma_start(gt8_sb[:], gt8_d[:])

        def stacked(dram, ntiles, width):
            return bass.AP(tensor=dram.tensor, offset=0,
                           ap=[[width, 128], [128 * width, ntiles], [1, width]])
        wk_sb = consts.tile([128, NJ, C], F8)
        wq_sb = consts.tile([128, NJ, C], F8)
        wv_sb = consts.tile([128, NJ, C], F8)
        pw_sb = consts.tile([128, NPJ, C], BF)
        nc.sync.dma_start(wk_sb[:], stacked(wk_d, NJ, C))
        nc.sync.dma_start(wq_sb[:], stacked(wq_d, NJ, C))
        nc.sync.dma_start(pw_sb[:], stacked(pw_d, NPJ, C))

        gns_sb, gnb_sb = cpk[:, 0:4], cpk[:, 4:8]
        bq_sb, bk_sb, pb_sb = cpk[:, 8:12], cpk[:, 12:16], cpk[:, 16:20]
        g8_sb = cpk[:, 20:28]

        # ---- GpSimd: memsets off the critical DVE/ACT paths ----
        zero_sb = consts.tile([128, 1], F32)
        gp_q.memset(zero_sb[:], 0.0)
        moff_sb = consts.tile([128, 1], F32)
        gp_q.memset(moff_sb[:], -EXP_OFF)
        gp_q.memset(vT2[:, :, :, CH:128], 1.0)   # ones block for row-sums

        # DVE: warm tile
        warm_sb = consts.tile([128, 512], BF)
        nc.vector.memset(warm_sb[:], 0.125)

        # ---- ACT queue: xbf j1/j3 DMAs, exp-table preload ----
        act_q.dma_start(xbf[:, 1, :], xbf_d[128:256, :])
        act_q.dma_start(xbf[:, 3, :], xbf_d[384:512, :])
        dume = consts.tile([128, 1], F32)
        nc.scalar.activation(dume[:], zero_sb[:], AFT.Exp, bias=zero_sb[:], scale=1.0)

        # ---- PE warmup: release the HAM clock gate before real matmuls ----
        def warm_mm(n=1):
            for _ in range(n):
                wps = kv_ps.tile([128, 512], F32, tag="kv", name="wps")
                nc.tensor.matmul(wps[:], warm_sb[:, 0:128], warm_sb[:],
                                 start=True, stop=True)

        def warm_dep(rhs):
            wps = kv_ps.tile([128, 512], F32, tag="kv", name="wpsd")
            nc.tensor.matmul(wps[:, 0:rhs.shape[-1]], warm_sb[:, 0:128], rhs,
                             start=True, stop=True)
        warm_mm(10)

        # ---- GroupNorm stats ----
        # ACT: sum(x^2) j0/j2 (Square+accum); DVE: sum(x) j0..j3 and
        # sum(x^2) j1/j3 via tensor_tensor_reduce.
        stats_sb = consts.tile([128, 8], F32)   # sum(x) j=0..3 | sum(x^2) j=0..3
        sq0 = work.tile([128, T], BF, tag="sqs", name="sq0")
        nc.scalar.activation(sq0[:], xbf[:, 0, :], AFT.Square,
                             accum_out=stats_sb[:, 4:5])
        sq2 = work.tile([128, T], BF, tag="sqs", name="sq2")
        nc.scalar.activation(sq2[:], xbf[:, 2, :], AFT.Square,
                             accum_out=stats_sb[:, 6:7])
        act_q.dma_start(wv_sb[:], stacked(wv_d, NJ, C))
        for j in (0, 2, 1, 3):
            nc.vector.tensor_reduce(stats_sb[:, j:j + 1], xbf[:, j, :], AXX, ADD)
            warm_dep(warm_sb[:])
        for j in (1, 3):
            sq = work.tile([128, T], BF, tag="sqs", name="sqv")
            nc.vector.tensor_mul(sq[:], xbf[:, j, :], xbf[:, j, :])
            nc.vector.tensor_reduce(stats_sb[:, 4 + j:5 + j], sq[:], AXX, ADD)
            warm_dep(sq[0:128, 0:512])

        # ---- batched group-reduce + Newton rsqrt + affine ----
        ps_st = kv_ps.tile([128, 512], F32, tag="kv")
        nc.tensor.matmul(ps_st[0:8, 0:8], g8_sb, stats_sb[:], start=True, stop=True)
        stg = work.tile([8, 16], F32, tag="stg")     # mean(0:4)|var(4:8)|t1|t2
        bcin = work.tile([8, 8], F32, tag="bcin")    # mean | rinv
        # g8 host values are pre-scaled by 1/GN_N, so ps_st already holds means
        nc.vector.tensor_copy(stg[:, 0:8], ps_st[0:8, 0:8])
        nc.vector.tensor_mul(stg[:, 8:12], stg[:, 0:4], stg[:, 0:4])
        nc.vector.scalar_tensor_tensor(stg[:, 4:8], stg[:, 8:12], -1.0, stg[:, 4:8],
                                       op0=MULT, op1=ADD)   # var
        nc.vector.tensor_scalar(bcin[:, 4:8], stg[:, 4:8], -0.5, 1.5, op0=MULT, op1=ADD)
        warm_mm(1)
        # Newton: y = y*(1.5 - 0.5*var*y^2)
        nc.vector.tensor_mul(stg[:, 8:12], stg[:, 4:8], bcin[:, 4:8])
        nc.vector.tensor_mul(stg[:, 12:16], stg[:, 8:12], bcin[:, 4:8])
        nc.vector.tensor_scalar(stg[:, 12:16], stg[:, 12:16], -0.5, 1.5, op0=MULT, op1=ADD)
        nc.vector.tensor_mul(bcin[:, 4:8], bcin[:, 4:8], stg[:, 12:16])
        nc.vector.tensor_copy(bcin[:, 0:4], stg[:, 0:4])
        ps_pp = kv_ps.tile([128, 512], F32, tag="kv")
        nc.tensor.matmul(ps_pp[0:128, 0:8], gt8_sb[:], bcin[:], start=True, stop=True)
        # dep-injected warm burst keeps the clock gate open through the chain
        nc.vector.tensor_copy(warm_sb[0:1, 0:1], bcin[0:1, 0:1])
        warm_mm(8)
        ab = consts.tile([128, 2, NJ], F32)   # scale | shift (x2 folded via host)
        nc.vector.tensor_mul(ab[:, 0, :], ps_pp[0:128, 4:8], gns_sb)
        t1b = work.tile([128, 4], F32, tag="t1b")
        nc.vector.tensor_mul(t1b[:], ps_pp[0:128, 0:4], ab[:, 0, :])
        nc.vector.tensor_sub(ab[:, 1, :], gnb_sb, t1b[:])

        # ---- xn8 (fp8) interleaved with k0 DoubleRow matmuls ----
        psk0 = kv_ps.tile([128, 512], F32, tag="kv", name="psk0")
        psk1 = kv_ps.tile([128, 512], F32, tag="kv", name="psk1")
        for j in range(NJ):
            eng = nc.vector if j < 2 else nc.gpsimd
            eng.tensor_scalar(xn8[:, j, :], xbf[:, j, :],
                              ab[:, 0, j:j + 1], ab[:, 1, j:j + 1],
                              op0=MULT, op1=ADD)
            if j % 2 == 1:
                jp = j - 1
                for n in range(2):
                    dr_mm([psk0, psk1][n][:],
                          wk_sb[:, jp:jp + 2, 0:128],
                          xn8[:, jp:jp + 2, 512 * n:512 * (n + 1)],
                          start=(jp == 0), stop=(jp == 2))
        for n in range(2):
            nc.vector.tensor_scalar(k_sb[:, 0, 512 * n:512 * (n + 1)], [psk0, psk1][n][:],
                                    KQ_DS, bk_sb[:, 0:1], op0=MULT, op1=ADD)

        # ---- filler units: 2 DoubleRow matmuls + 1 evac each ----
        def kq_unit(w_sb, b_sb, dst, m, n):
            def emit():
                ps = kv_ps.tile([128, 512], F32, tag="kv", name="ps_kq")
                for jp in (0, 2):
                    dr_mm(ps[:],
                          w_sb[:, jp:jp + 2, 128 * m:128 * (m + 1)],
                          xn8[:, jp:jp + 2, 512 * n:512 * (n + 1)],
                          start=(jp == 0), stop=(jp == 2))
                nc.vector.tensor_scalar(dst[:, m, 512 * n:512 * (n + 1)], ps[:],
                                        KQ_DS, b_sb[:, m:m + 1], op0=MULT, op1=ADD)
            return emit

        def v_unit(tm):
            def emit():
                ps = kv_ps.tile([128, 512], F32, tag="kv", name="ps_v")
                for jp in (0, 2):
                    dr_mm(ps[:],
                          xn8[:, jp:jp + 2, 128 * tm:128 * (tm + 1)],
                          wv_sb[:, jp:jp + 2, :],
                          start=(jp == 0), stop=(jp == 2))
                nc.vector.tensor_scalar(
                    vT2[:, tm, :, 0:CH],
                    ps[:].rearrange("p (h c) -> p h c", c=CH),
                    KQ_DS, None, op0=MULT)
            return emit

        # proj helpers: psum accumulates pw@a + x via [pw | I] chunks.
        # identity chunk for out-tile m is jj=4+m (block-diagonal).
        def proj_chunks(ps, m, nh, chunks, last):
            for jj in chunks:
                rhs = (xbf if jj >= NJ else a_sb)
                nc.tensor.matmul(ps,
                                 pw_sb[:, jj, 128 * m:128 * (m + 1)],
                                 rhs[:, jj % NJ, 512 * nh:512 * (nh + 1)],
                                 start=(jj == chunks[0]), stop=(jj == last))

        osb_ct = [0]

        def proj_evac_dma(ps, m, nh, on_act=False):
            osb = outp.tile([128, 512], F32, tag="osb", name=f"osb{osb_ct[0]}")
            osb_ct[0] += 1
            if on_act:
                nc.scalar.activation(osb[:], ps, AFT.Identity,
                                     bias=pb_sb[:, m:m + 1], scale=1.0)
            else:
                nc.vector.tensor_scalar(osb[:], ps, pb_sb[:, m:m + 1], None, op0=ADD)
            eng = act_q if on_act else nc.sync
            eng.dma_start(out_d[128 * m:128 * (m + 1), 512 * nh:512 * (nh + 1)], osb[:])

        def proj_unit_full(m):
            # complete nh=0 out-tile (a3 ready after block 3's evac)
            def emit():
                ps = kv_ps.tile([128, 512], F32, tag="kv", name="ps_pj")
                proj_chunks(ps[:], m, 0, [4 + m, 0, 1, 2, 3], 3)
                proj_evac_dma(ps[:], m, 0)
            return emit

        # fillers: (deadline_block, emit). v units feed block-0 AV drains
        # in-stream; k/q for pair p due before block p (n=0) / 4+p (n=1).
        fillers = [
            (1, v_unit(0)), (1, v_unit(1)), (1, v_unit(2)), (1, v_unit(3)),
            (1, kq_unit(wk_sb, bk_sb, k_sb, 1, 0)),
            (1, kq_unit(wk_sb, bk_sb, k_sb, 1, 1)),
            (1, v_unit(4)), (1, v_unit(5)),
            (1, kq_unit(wq_sb, bq_sb, q_sb, 1, 0)),
            (1, v_unit(6)), (1, v_unit(7)),
            (2, kq_unit(wk_sb, bk_sb, k_sb, 2, 0)),
            (2, kq_unit(wk_sb, bk_sb, k_sb, 2, 1)),
            (2, kq_unit(wq_sb, bq_sb, q_sb, 2, 0)),
            (3, kq_unit(wk_sb, bk_sb, k_sb, 3, 0)),
            (3, kq_unit(wk_sb, bk_sb, k_sb, 3, 1)),
            (3, kq_unit(wq_sb, bq_sb, q_sb, 3, 0)),
            (4, kq_unit(wq_sb, bq_sb, q_sb, 0, 1)),
            (5, kq_unit(wq_sb, bq_sb, q_sb, 1, 1)),
            (6, kq_unit(wq_sb, bq_sb, q_sb, 2, 1)),
            (7, kq_unit(wq_sb, bq_sb, q_sb, 3, 1)),
        ]
        # nh=0 proj out-tiles: fully computable + shippable from block 4 on
        late = [(36, proj_unit_full(0)), (40, proj_unit_full(1)),
                (44, proj_unit_full(2)), (48, proj_unit_full(3))]
        # nh=1 m0/m1 pre-chunks (x + a0..a2) held in the kv banks to the tail
        psn1 = [None, None]

        def n1_pre(m):
            def emit():
                psn1[m] = kv_ps.tile([128, 512], F32, tag="kv", name=f"psn1_{m}")
                proj_chunks(psn1[m][:], m, 1, [4 + m, 0, 1, 2], -1)
            return emit


        def pump(slot, big=False):
            if fillers:
                fillers.pop(0)[1]()
            elif big and late and slot >= late[0][0]:
                late.pop(0)[1]()
            elif slot < 62:
                warm_mm(1)

        # ---- q0 n=0 ----
        kq_unit(wq_sb, bq_sb, q_sb, 0, 0)()

        # ---- attention: n-OUTER blocks; AV = fp8 DoubleRow over sm-pairs ----
        def emit_qk(p, n, sm):
            psw = qk_ps.tile([128, T], F32, tag="qk", name="psw")
            nc.tensor.matmul(psw[:, 0:512],
                             k_sb[0:64, p, 128 * sm:128 * (sm + 1)],
                             q_sb[0:64, p, 512 * n:512 * (n + 1)],
                             start=True, stop=True, tile_position=(0, 0))
            nc.tensor.matmul(psw[:, 512:1024],
                             k_sb[64:128, p, 128 * sm:128 * (sm + 1)],
                             q_sb[64:128, p, 512 * n:512 * (n + 1)],
                             start=True, stop=True, tile_position=(64, 0))
            nc.scalar.activation(ew_ring[:, sm, :, :],
                                 psw[:].rearrange("p (u t) -> p u t", u=2),
                                 AFT.Exp, bias=moff_sb[:], scale=EXP_SCALE)

        def emit_evac(p, n, psa):
            # custom-DVE ops need partition-0 based SBUF APs on HW
            for u in range(2):
                rs = rinvp.tile([64, 512], F32, tag="rs", name="rs")
                nc.vector.tensor_copy(rs[:], psa[u][64:128, :])
                rinv = rinvp.tile([64, 512], F32, tag="rinv", name="rinv")
                nc.vector.reciprocal_approx_fast(rinv[:], rs[:])
                nc.vector.tensor_mul(a_sb[64 * u:64 * (u + 1), p, 512 * n:512 * (n + 1)],
                                     psa[u][0:CH, :], rinv[:])

        blocks = [(p, n) for n in range(2) for p in range(NJ)]
        bpsa = {}
        pend = []

        def drain(auto_evac=True):
            bi, p, n, sp = pend.pop(0)
            for u in range(2):
                dr_mm(bpsa[bi][u][:],
                      vT2[:, 2 * sp:2 * sp + 2, 2 * p + u, :],
                      ew_ring[:, 2 * sp:2 * sp + 2, u, :],
                      start=(sp == 0), stop=(sp == 3))
            if sp == 3 and auto_evac:
                emit_evac(p, n, bpsa[bi])
                return True
            return False

        for bi, (p, n) in enumerate(blocks):
            while fillers and fillers[0][0] <= bi:
                fillers.pop(0)[1]()
            psa0 = av_ps.tile([128, 512], F32, tag="av", name="psa0")
            psa1 = av_ps.tile([128, 512], F32, tag="av", name="psa1")
            bpsa[bi] = [psa0, psa1]
            for sp in range(4):
                slot = 8 * bi + 2 * sp
                emit_qk(p, n, 2 * sp)
                pump(slot)
                emit_qk(p, n, 2 * sp + 1)
                pend.append((bi, p, n, sp))
                evd = False
                if len(pend) > 2:
                    evd = drain()
                if not evd:
                    pump(slot + 1, big=True)

        # ---- tail: nh=1 only ----
        drain()                      # AV(last, sp=2)
        lbi, lp, ln_, lsp = pend[0]
        drain(auto_evac=False)       # AV(last, sp=3)
        n1_pre(0)()
        n1_pre(1)()
        # m2/m3 nh=1 pre-chunks in the qk psum banks freed by the last exps
        psq2 = qk_ps.tile([128, T], F32, tag="qk", name="psq2")
        psq3 = qk_ps.tile([128, T], F32, tag="qk", name="psq3")
        proj_chunks(psq2[:, 0:512], 2, 1, [6, 0, 1, 2], -1)
        proj_chunks(psq3[:, 0:512], 3, 1, [7, 0, 1, 2], -1)
        emit_evac(lp, ln_, bpsa[lbi])
        finals = [psn1[0][:], psn1[1][:], psq2[:, 0:512], psq3[:, 0:512]]
        for m in range(4):
            nc.tensor.matmul(finals[m],
                             pw_sb[:, 3, 128 * m:128 * (m + 1)],
                             a_sb[:, 3, 512:1024],
                             start=False, stop=True)
        # evacuations + DMA issues alternate Sync / ACT queues
        for m in range(4):
            proj_evac_dma(finals[m], m, 1, on_act=(m % 2 == 1))

    nc.compile()
    return nc


_NC_CACHE = {}


def get_nc():
    if "nc" not in _NC_CACHE:
        _NC_CACHE["nc"] = build_graph()
    return _NC_CACHE["nc"]


def make_in_maps(x, norm_scale, norm_bias, qkv_w, qkv_b, proj_w, proj_b):
    x = np.asarray(x, dtype=np.float32)
    B = x.shape[0]
    qr = np.asarray(qkv_w, np.float32).reshape(H, 3, CH, C)
    wq = np.ascontiguousarray(qr[:, 0].reshape(C, C).T * WS).astype(F8_NP)
    wk = np.ascontiguousarray(qr[:, 1].reshape(C, C).T * WS).astype(F8_NP)
    wv = np.ascontiguousarray(qr[:, 2].reshape(C, C).T * WS).astype(F8_NP)
    br = np.asarray(qkv_b, np.float32).reshape(H, 3, CH)
    bq = np.ascontiguousarray(br[:, 0].reshape(C))
    bk = np.ascontiguousarray(br[:, 1].reshape(C))
    bv = np.ascontiguousarray(br[:, 2].reshape(C))
    pw_f = np.asarray(proj_w, np.float32)
    # residual folded in: [pw | I] so proj psum accumulates pw@a + x
    pw2 = np.ascontiguousarray(
        np.concatenate([pw_f.T, np.eye(C, dtype=np.float32)], axis=0)).astype(BF_NP)
    # v bias folded through proj: h = pw @ (a + bv) + pb = pw @ a + (pw@bv + pb)
    pb2 = np.asarray(proj_b, np.float32) + pw_f @ bv
    g8 = np.zeros((128, 8), np.float32)
    g8[np.arange(128), np.arange(128) // 16] = 1.0
    gt8 = np.ascontiguousarray(g8.T)
    g8s = g8 * np.float32(1.0 / GN_N)   # fold the 1/N of the group mean into g8
    cpack = np.zeros((128, 28), np.float32)
    # XS folded into the groupnorm affine (xn8 = 2*xn)
    cpack[:, 0:4] = (np.asarray(norm_scale, np.float32) * XS).reshape(NJ, 128).T
    cpack[:, 4:8] = (np.asarray(norm_bias, np.float32) * XS).reshape(NJ, 128).T
    cpack[:, 8:12] = bq.reshape(NJ, 128).T
    cpack[:, 12:16] = bk.reshape(NJ, 128).T
    cpack[:, 16:20] = pb2.reshape(NJ, 128).T
    cpack[:, 20:28] = g8s
    shared = dict(wq=wq, wk=wk, wv=wv, pw=pw2,
                  cpack=np.ascontiguousarray(cpack),
                  gt8=gt8)
    in_maps = []
    for i in range(B):
        m = dict(shared)
        m["xbf"] = np.ascontiguousarray(x[i].reshape(C, T).astype(BF_NP))
        in_maps.append(m)
    return in_maps


def kernel(x, norm_scale, norm_bias, qkv_w, qkv_b, proj_w, proj_b):
    x = np.asarray(x, dtype=np.float32)
    B, Cc, Hh, Ww = x.shape
    nc = get_nc()
    in_maps = make_in_maps(x, norm_scale, norm_bias, qkv_w, qkv_b, proj_w, proj_b)
    res = run_bass_kernel_spmd(nc, in_maps, core_ids=list(range(B)))
    out = np.stack([res.results[i]["out"] for i in range(B)])
    return out.reshape(B, Cc, Hh, Ww).astype(np.float32)


# revision 12
# speedup vs baseline: 1.1934x; 1.1934x over previous
"""AttentionBlock (GroupNorm32 + QKV 8-head attention + proj + residual) on 8 TRN2 NeuronCores.

Sharding: pure data-parallel over batch B=8 - one batch element per core.

v2 design (fp8 + restructured schedule). Per-core engine walls: ACT must run
64 softmax Exps of 1024 elems (~73us busy - the only engine with exp); the PE
matmul stream is cut below that wall with fp8 DoubleRow (contraction 256/pass)
for the qkv projections, v, and AV (q/k/QK and the x-residual stay bf16 for
accuracy; numerically validated ~5.7e-3 total vs 2e-2 budget):

  - front: input DMAs issued on BOTH hardware DGE queues (Sync + ACT);
    GN stats split DVE (4 sums + 2 sumsq via tensor_tensor_reduce) / ACT
    (2 squares); memsets (ones block, exp bias) on the idle GpSimd; a
    warm-matmul burst holds the HAM clock gate; first Exp at ~16us vs 28.
  - attention: blocks ordered n-OUTER ((p,0) for all p, then (p,1)) so all
    nh=0 attention outputs exist by block 3; proj out-tiles for nh=0 are
    fully contracted + evacuated + DMA'd DURING blocks 4-6 - half the output
    ships mid-attention. exp writes fp8 ew into a persistent 8-slot ring
    (one tensor, so DoubleRow AV can take a [128,2,512] sm-pair AP).
  - AV: stationary [v | ones] fp8 per (sm,h); DoubleRow contracts an
    sm-PAIR per matmul; psum rows 64:128 hold softmax row-sums for free.
  - fillers: kq/v units are naturally micro in fp8 (2 DoubleRow matmuls);
    pumped between sm's (with per-block deadlines) so neither the QK->exp
    chain nor the AV v-dependency ever stalls.
  - tail: only the nh=1 half remains: a3 evac, m2/m3 runs in the freed qk
    psum banks, finals, evacs, DMA issues split across Sync/ACT queues.
"""

import numpy as np
import ml_dtypes
from contextlib import ExitStack

import concourse.bass as bass
import concourse.tile as tile
from concourse import bacc, mybir
from concourse.bass_utils import run_bass_kernel_spmd

F32 = mybir.dt.float32
BF = mybir.dt.bfloat16
F8 = mybir.dt.float8e4
MULT = mybir.AluOpType.mult
ADD = mybir.AluOpType.add
AFT = mybir.ActivationFunctionType
AXX = mybir.AxisListType.X
DR = mybir.MatmulPerfMode.DoubleRow

C, T, H, CH = 512, 1024, 8, 64
NJ = C // 128          # 4 c-tiles
NTM = T // 128         # 8 t-tiles
NPJ = 8                # proj contraction chunks: [pw | I]
EXP_SCALE = float(CH) ** -0.5  # folded (q*s)*(k*s) scale, s = ch**-0.25
EXP_OFF = 4.0                  # exp(l - 4): keeps fp8 ew in e4m3 range
GN_N = 16 * T          # elements per group
WS = 8.0               # host weight scale (w8 = fp8(w*8))
XS = 2.0               # xn scale (xn8 = fp8(2*xn), folded into gn affine)
KQ_DS = 1.0 / (WS * XS)  # psum -> true k/q/v

BF_NP = ml_dtypes.bfloat16
F8_NP = ml_dtypes.float8_e4m3


def build_graph(enable_asserts: bool = False, use_act_dma: bool = True,
                use_gp_memset: bool = True, use_dr: bool = True):
    nc = bacc.Bacc(
        "TRN2",
        target_bir_lowering=False,
        debug=False,
        enable_asserts=enable_asserts,
    )
    act_q = nc.scalar if use_act_dma else nc.sync
    gp_q = nc.gpsimd if use_gp_memset else nc.vector

    def dr_mm(out, lhsT3, rhs3, start, stop):
        # one fp8 DoubleRow matmul over a k-tile pair, or two plain matmuls
        if use_dr:
            nc.tensor.matmul(out, lhsT3, rhs3, start=start, stop=stop,
                             perf_mode=DR)
        else:
            nc.tensor.matmul(out, lhsT3[:, 0, :], rhs3[:, 0, :],
                             start=start, stop=False)
            nc.tensor.matmul(out, lhsT3[:, 1, :], rhs3[:, 1, :],
                             start=False, stop=stop)
    xbf_d = nc.dram_tensor("xbf", [C, T], BF, kind="ExternalInput").ap()
    wq_d = nc.dram_tensor("wq", [C, C], F8, kind="ExternalInput").ap()
    wk_d = nc.dram_tensor("wk", [C, C], F8, kind="ExternalInput").ap()
    wv_d = nc.dram_tensor("wv", [C, C], F8, kind="ExternalInput").ap()
    pw_d = nc.dram_tensor("pw", [2 * C, C], BF, kind="ExternalInput").ap()
    cp_d = nc.dram_tensor("cpack", [128, 28], F32, kind="ExternalInput").ap()
    gt8_d = nc.dram_tensor("gt8", [8, 128], F32, kind="ExternalInput").ap()
    out_d = nc.dram_tensor("out", [C, T], F32, kind="ExternalOutput").ap()

    with tile.TileContext(nc) as tc, ExitStack() as ctx:
        consts = ctx.enter_context(tc.tile_pool(name="consts", bufs=1))
        bigs = ctx.enter_context(tc.tile_pool(name="bigs", bufs=1))
        work = ctx.enter_context(tc.tile_pool(name="work", bufs=4))
        rinvp = ctx.enter_context(tc.tile_pool(name="rinvp", bufs=4))
        outp = ctx.enter_context(tc.tile_pool(name="outp", bufs=4))
        qk_ps = ctx.enter_context(tc.tile_pool(name="qk_ps", bufs=2, space="PSUM"))
        kv_ps = ctx.enter_context(tc.tile_pool(name="kv_ps", bufs=2, space="PSUM"))
        av_ps = ctx.enter_context(tc.tile_pool(name="av_ps", bufs=2, space="PSUM"))

        # ---- persistent sbuf tensors ----
        xbf = bigs.tile([128, NJ, T], BF)       # bf16 x (GN stats + residual)
        xn8 = bigs.tile([128, NJ, T], F8)       # 2*groupnormed x (fp8)
        q_sb = bigs.tile([128, NJ, T], BF)      # q rows (head-major)
        k_sb = bigs.tile([128, NJ, T], BF)      # k rows (head-major)
        vT2 = bigs.tile([128, NTM, H, 128], F8)  # [v | ones] per (tm, head)
        ew_ring = bigs.tile([128, NTM, 2, 512], F8)  # exp ring: (sm%8, u, t)
        a_sb = bigs.tile([128, NJ, T], BF)      # normalized attention output

        # ---- DMAs split across both HWDGE queues (Sync + ACT) ----
        nc.sync.dma_start(xbf[:, 0, :], xbf_d[0:128, :])
        nc.sync.dma_start(xbf[:, 2, :], xbf_d[256:384, :])
        cpk = consts.tile([128, 28], F32)
        nc.sync.dma_start(cpk[:], cp_d[:])
        gt8_sb = consts.tile([8, 128], F32)
        nc.sync.dma_start(gt8_sb[:], gt8_d[:])

        def stacked(dram, ntiles, width):
            return bass.AP(tensor=dram.tensor, offset=0,
                           ap=[[width, 128], [128 * width, ntiles], [1, width]])
        wk_sb = consts.tile([128, NJ, C], F8)
        wq_sb = consts.tile([128, NJ, C], F8)
        wv_sb = consts.tile([128, NJ, C], F8)
        pw_sb = consts.tile([128, NPJ, C], BF)
        nc.sync.dma_start(wk_sb[:], stacked(wk_d, NJ, C))
        nc.sync.dma_start(wq_sb[:], stacked(wq_d, NJ, C))
        nc.sync.dma_start(pw_sb[:], stacked(pw_d, NPJ, C))

        gns_sb, gnb_sb = cpk[:, 0:4], cpk[:, 4:8]
        bq_sb, bk_sb, pb_sb = cpk[:, 8:12], cpk[:, 12:16], cpk[:, 16:20]
        g8_sb = cpk[:, 20:28]

        # ---- GpSimd: memsets off the critical DVE/ACT paths ----
        zero_sb = consts.tile([128, 1], F32)
        gp_q.memset(zero_sb[:], 0.0)
        moff_sb = consts.tile([128, 1], F32)
        gp_q.memset(moff_sb[:], -EXP_OFF)
        gp_q.memset(vT2[:, :, :, CH:128], 1.0)   # ones block for row-sums

        # DVE: warm tile
        warm_sb = consts.tile([128, 512], BF)
        nc.vector.memset(warm_sb[:], 0.125)

        # ---- ACT queue: xbf j1/j3 DMAs, exp-table preload ----
        act_q.dma_start(xbf[:, 1, :], xbf_d[128:256, :])
        act_q.dma_start(xbf[:, 3, :], xbf_d[384:512, :])
        dume = consts.tile([128, 1], F32)
        nc.scalar.activation(dume[:], zero_sb[:], AFT.Exp, bias=zero_sb[:], scale=1.0)

        # ---- PE warmup: release the HAM clock gate before real matmuls ----
        def warm_mm(n=1):
            for _ in range(n):
                wps = kv_ps.tile([128, 512], F32, tag="kv", name="wps")
                nc.tensor.matmul(wps[:], warm_sb[:, 0:128], warm_sb[:],
                                 start=True, stop=True)

        def warm_dep(rhs):
            wps = kv_ps.tile([128, 512], F32, tag="kv", name="wpsd")
            nc.tensor.matmul(wps[:, 0:rhs.shape[-1]], warm_sb[:, 0:128], rhs,
                             start=True, stop=True)
        warm_mm(10)

        # ---- GroupNorm stats ----
        # ACT: sum(x^2) j0/j2 (Square+accum); DVE: sum(x) j0..j3 and
        # sum(x^2) j1/j3 via tensor_tensor_reduce.
        stats_sb = consts.tile([128, 8], F32)   # sum(x) j=0..3 | sum(x^2) j=0..3
        sq0 = work.tile([128, T], BF, tag="sqs", name="sq0")
        nc.scalar.activation(sq0[:], xbf[:, 0, :], AFT.Square,
                             accum_out=stats_sb[:, 4:5])
        sq2 = work.tile([128, T], BF, tag="sqs", name="sq2")
        nc.scalar.activation(sq2[:], xbf[:, 2, :], AFT.Square,
                             accum_out=stats_sb[:, 6:7])
        act_q.dma_start(wv_sb[:], stacked(wv_d, NJ, C))
        for j in (0, 2, 1, 3):
            nc.vector.tensor_reduce(stats_sb[:, j:j + 1], xbf[:, j, :], AXX, ADD)
            warm_dep(warm_sb[:])
        for j in (1, 3):
            sq = work.tile([128, T], BF, tag="sqs", name="sqv")
            nc.vector.tensor_mul(sq[:], xbf[:, j, :], xbf[:, j, :])
            nc.vector.tensor_reduce(stats_sb[:, 4 + j:5 + j], sq[:], AXX, ADD)
            warm_dep(sq[0:128, 0:512])

        # ---- batched group-reduce + Newton rsqrt + affine ----
        ps_st = kv_ps.tile([128, 512], F32, tag="kv")
        nc.tensor.matmul(ps_st[0:8, 0:8], g8_sb, stats_sb[:], start=True, stop=True)
        stg = work.tile([8, 16], F32, tag="stg")     # mean(0:4)|var(4:8)|t1|t2
        bcin = work.tile([8, 8], F32, tag="bcin")    # mean | rinv
        # g8 host values are pre-scaled by 1/GN_N, so ps_st already holds means
        nc.vector.tensor_copy(stg[:, 0:8], ps_st[0:8, 0:8])
        nc.vector.tensor_mul(stg[:, 8:12], stg[:, 0:4], stg[:, 0:4])
        nc.vector.scalar_tensor_tensor(stg[:, 4:8], stg[:, 8:12], -1.0, stg[:, 4:8],
                                       op0=MULT, op1=ADD)   # var
        nc.vector.tensor_scalar(bcin[:, 4:8], stg[:, 4:8], -0.5, 1.5, op0=MULT, op1=ADD)
        warm_mm(1)
        # Newton: y = y*(1.5 - 0.5*var*y^2)
        nc.vector.tensor_mul(stg[:, 8:12], stg[:, 4:8], bcin[:, 4:8])
        nc.vector.tensor_mul(stg[:, 12:16], stg[:, 8:12], bcin[:, 4:8])
        nc.vector.tensor_scalar(stg[:, 12:16], stg[:, 12:16], -0.5, 1.5, op0=MULT, op1=ADD)
        nc.vector.tensor_mul(bcin[:, 4:8], bcin[:, 4:8], stg[:, 12:16])
        nc.vector.tensor_copy(bcin[:, 0:4], stg[:, 0:4])
        ps_pp = kv_ps.tile([128, 512], F32, tag="kv")
        nc.tensor.matmul(ps_pp[0:128, 0:8], gt8_sb[:], bcin[:], start=True, stop=True)
        # dep-injected warm burst keeps the clock gate open through the chain
        nc.vector.tensor_copy(warm_sb[0:1, 0:1], bcin[0:1, 0:1])
        warm_mm(8)
        ab = consts.tile([128, 2, NJ], F32)   # scale | shift (x2 folded via host)
        nc.vector.tensor_mul(ab[:, 0, :], ps_pp[0:128, 4:8], gns_sb)
        t1b = work.tile([128, 4], F32, tag="t1b")
        nc.vector.tensor_mul(t1b[:], ps_pp[0:128, 0:4], ab[:, 0, :])
        nc.vector.tensor_sub(ab[:, 1, :], gnb_sb, t1b[:])

        # ---- xn8 (fp8) interleaved with k0 DoubleRow matmuls ----
        psk0 = kv_ps.tile([128, 512], F32, tag="kv", name="psk0")
        psk1 = kv_ps.tile([128, 512], F32, tag="kv", name="psk1")
        for j in range(NJ):
            nc.vector.tensor_scalar(xn8[:, j, :], xbf[:, j, :],
                                    ab[:, 0, j:j + 1], ab[:, 1, j:j + 1],
                                    op0=MULT, op1=ADD)
            if j % 2 == 1:
                jp = j - 1
                for n in range(2):
                    dr_mm([psk0, psk1][n][:],
                          wk_sb[:, jp:jp + 2, 0:128],
                          xn8[:, jp:jp + 2, 512 * n:512 * (n + 1)],
                          start=(jp == 0), stop=(jp == 2))
        for n in range(2):
            nc.vector.tensor_scalar(k_sb[:, 0, 512 * n:512 * (n + 1)], [psk0, psk1][n][:],
                                    KQ_DS, bk_sb[:, 0:1], op0=MULT, op1=ADD)

        # ---- filler units: 2 DoubleRow matmuls + 1 evac each ----
        def kq_unit(w_sb, b_sb, dst, m, n):
            def emit():
                ps = kv_ps.tile([128, 512], F32, tag="kv", name="ps_kq")
                for jp in (0, 2):
                    dr_mm(ps[:],
                          w_sb[:, jp:jp + 2, 128 * m:128 * (m + 1)],
                          xn8[:, jp:jp + 2, 512 * n:512 * (n + 1)],
                          start=(jp == 0), stop=(jp == 2))
                nc.vector.tensor_scalar(dst[:, m, 512 * n:512 * (n + 1)], ps[:],
                                        KQ_DS, b_sb[:, m:m + 1], op0=MULT, op1=ADD)
            return emit

        def v_unit(tm):
            def emit():
                ps = kv_ps.tile([128, 512], F32, tag="kv", name="ps_v")
                for jp in (0, 2):
                    dr_mm(ps[:],
                          xn8[:, jp:jp + 2, 128 * tm:128 * (tm + 1)],
                          wv_sb[:, jp:jp + 2, :],
                          start=(jp == 0), stop=(jp == 2))
                nc.vector.tensor_scalar(
                    vT2[:, tm, :, 0:CH],
                    ps[:].rearrange("p (h c) -> p h c", c=CH),
                    KQ_DS, None, op0=MULT)
            return emit

        # proj helpers: psum accumulates pw@a + x via [pw | I] chunks.
        # identity chunk for out-tile m is jj=4+m (block-diagonal).
        def proj_chunks(ps, m, nh, chunks, last):
            for jj in chunks:
                rhs = (xbf if jj >= NJ else a_sb)
                nc.tensor.matmul(ps,
                                 pw_sb[:, jj, 128 * m:128 * (m + 1)],
                                 rhs[:, jj % NJ, 512 * nh:512 * (nh + 1)],
                                 start=(jj == chunks[0]), stop=(jj == last))

        osb_ct = [0]

        def proj_evac_dma(ps, m, nh, on_act=False):
            osb = outp.tile([128, 512], F32, tag="osb", name=f"osb{osb_ct[0]}")
            osb_ct[0] += 1
            if on_act:
                nc.scalar.activation(osb[:], ps, AFT.Identity,
                                     bias=pb_sb[:, m:m + 1], scale=1.0)
            else:
                nc.vector.tensor_scalar(osb[:], ps, pb_sb[:, m:m + 1], None, op0=ADD)
            eng = act_q if on_act else nc.sync
            eng.dma_start(out_d[128 * m:128 * (m + 1), 512 * nh:512 * (nh + 1)], osb[:])

        def proj_unit_full(m):
            # complete nh=0 out-tile (a3 ready after block 3's evac)
            def emit():
                ps = kv_ps.tile([128, 512], F32, tag="kv", name="ps_pj")
                proj_chunks(ps[:], m, 0, [4 + m, 0, 1, 2, 3], 3)
                proj_evac_dma(ps[:], m, 0)
            return emit

        # fillers: (deadline_block, emit). v units feed block-0 AV drains
        # in-stream; k/q for pair p due before block p (n=0) / 4+p (n=1).
        fillers = [
            (1, v_unit(0)), (1, v_unit(1)), (1, v_unit(2)), (1, v_unit(3)),
            (1, kq_unit(wk_sb, bk_sb, k_sb, 1, 0)),
            (1, kq_unit(wk_sb, bk_sb, k_sb, 1, 1)),
            (1, v_unit(4)), (1, v_unit(5)),
            (1, kq_unit(wq_sb, bq_sb, q_sb, 1, 0)),
            (1, v_unit(6)), (1, v_unit(7)),
            (2, kq_unit(wk_sb, bk_sb, k_sb, 2, 0)),
            (2, kq_unit(wk_sb, bk_sb, k_sb, 2, 1)),
            (2, kq_unit(wq_sb, bq_sb, q_sb, 2, 0)),
            (3, kq_unit(wk_sb, bk_sb, k_sb, 3, 0)),
            (3, kq_unit(wk_sb, bk_sb, k_sb, 3, 1)),
            (3, kq_unit(wq_sb, bq_sb, q_sb, 3, 0)),
            (4, kq_unit(wq_sb, bq_sb, q_sb, 0, 1)),
            (5, kq_unit(wq_sb, bq_sb, q_sb, 1, 1)),
            (6, kq_unit(wq_sb, bq_sb, q_sb, 2, 1)),
            (7, kq_unit(wq_sb, bq_sb, q_sb, 3, 1)),
        ]
        # nh=0 proj out-tiles: fully computable + shippable from block 4 on
        late = [(36, proj_unit_full(0)), (40, proj_unit_full(1)),
                (44, proj_unit_full(2)), (48, proj_unit_full(3))]
        # nh=1 m0/m1 pre-chunks (x + a0..a2) held in the kv banks to the tail
        psn1 = [None, None]

        def n1_pre(m):
            def emit():
                psn1[m] = kv_ps.tile([128, 512], F32, tag="kv", name=f"psn1_{m}")
                proj_chunks(psn1[m][:], m, 1, [4 + m, 0, 1, 2], -1)
            return emit


        def pump(slot, big=False):
            if fillers:
                fillers.pop(0)[1]()
            elif big and late and slot >= late[0][0]:
                late.pop(0)[1]()
            elif slot < 62:
                warm_mm(1)

        # ---- q0 n=0 ----
        kq_unit(wq_sb, bq_sb, q_sb, 0, 0)()

        # ---- attention: n-OUTER blocks; AV = fp8 DoubleRow over sm-pairs ----
        def emit_qk(p, n, sm):
            psw = qk_ps.tile([128, T], F32, tag="qk", name="psw")
            nc.tensor.matmul(psw[:, 0:512],
                             k_sb[0:64, p, 128 * sm:128 * (sm + 1)],
                             q_sb[0:64, p, 512 * n:512 * (n + 1)],
                             start=True, stop=True, tile_position=(0, 0))
            nc.tensor.matmul(psw[:, 512:1024],
                             k_sb[64:128, p, 128 * sm:128 * (sm + 1)],
                             q_sb[64:128, p, 512 * n:512 * (n + 1)],
                             start=True, stop=True, tile_position=(64, 0))
            nc.scalar.activation(ew_ring[:, sm, :, :],
                                 psw[:].rearrange("p (u t) -> p u t", u=2),
                                 AFT.Exp, bias=moff_sb[:], scale=EXP_SCALE)

        def emit_evac(p, n, psa):
            # custom-DVE ops need partition-0 based SBUF APs on HW
            for u in range(2):
                rs = rinvp.tile([64, 512], F32, tag="rs", name="rs")
                nc.vector.tensor_copy(rs[:], psa[u][64:128, :])
                rinv = rinvp.tile([64, 512], F32, tag="rinv", name="rinv")
                nc.vector.reciprocal_approx_fast(rinv[:], rs[:])
                nc.vector.tensor_mul(a_sb[64 * u:64 * (u + 1), p, 512 * n:512 * (n + 1)],
                                     psa[u][0:CH, :], rinv[:])

        blocks = [(p, n) for n in range(2) for p in range(NJ)]
        bpsa = {}
        pend = []

        def drain(auto_evac=True):
            bi, p, n, sp = pend.pop(0)
            for u in range(2):
                dr_mm(bpsa[bi][u][:],
                      vT2[:, 2 * sp:2 * sp + 2, 2 * p + u, :],
                      ew_ring[:, 2 * sp:2 * sp + 2, u, :],
                      start=(sp == 0), stop=(sp == 3))
            if sp == 3 and auto_evac:
                emit_evac(p, n, bpsa[bi])
                return True
            return False

        for bi, (p, n) in enumerate(blocks):
            while fillers and fillers[0][0] <= bi:
                fillers.pop(0)[1]()
            psa0 = av_ps.tile([128, 512], F32, tag="av", name="psa0")
            psa1 = av_ps.tile([128, 512], F32, tag="av", name="psa1")
            bpsa[bi] = [psa0, psa1]
            for sp in range(4):
                slot = 8 * bi + 2 * sp
                emit_qk(p, n, 2 * sp)
                pump(slot)
                emit_qk(p, n, 2 * sp + 1)
                pend.append((bi, p, n, sp))
                evd = False
                if len(pend) > 2:
                    evd = drain()
                if not evd:
                    pump(slot + 1, big=True)

        # ---- tail: nh=1 only ----
        drain()                      # AV(last, sp=2)
        lbi, lp, ln_, lsp = pend[0]
        drain(auto_evac=False)       # AV(last, sp=3)
        n1_pre(0)()
        n1_pre(1)()
        # m2/m3 nh=1 pre-chunks in the qk psum banks freed by the last exps
        psq2 = qk_ps.tile([128, T], F32, tag="qk", name="psq2")
        psq3 = qk_ps.tile([128, T], F32, tag="qk", name="psq3")
        proj_chunks(psq2[:, 0:512], 2, 1, [6, 0, 1, 2], -1)
        proj_chunks(psq3[:, 0:512], 3, 1, [7, 0, 1, 2], -1)
        emit_evac(lp, ln_, bpsa[lbi])
        finals = [psn1[0][:], psn1[1][:], psq2[:, 0:512], psq3[:, 0:512]]
        for m in range(4):
            nc.tensor.matmul(finals[m],
                             pw_sb[:, 3, 128 * m:128 * (m + 1)],
                             a_sb[:, 3, 512:1024],
                             start=False, stop=True)
        # evacuations + DMA issues alternate Sync / ACT queues
        for m in range(4):
            proj_evac_dma(finals[m], m, 1, on_act=(m % 2 == 1))

    nc.compile()
    return nc


_NC_CACHE = {}


def get_nc():
    if "nc" not in _NC_CACHE:
        _NC_CACHE["nc"] = build_graph()
    return _NC_CACHE["nc"]


def make_in_maps(x, norm_scale, norm_bias, qkv_w, qkv_b, proj_w, proj_b):
    x = np.asarray(x, dtype=np.float32)
    B = x.shape[0]
    qr = np.asarray(qkv_w, np.float32).reshape(H, 3, CH, C)
    wq = np.ascontiguousarray(qr[:, 0].reshape(C, C).T * WS).astype(F8_NP)
    wk = np.ascontiguousarray(qr[:, 1].reshape(C, C).T * WS).astype(F8_NP)
    wv = np.ascontiguousarray(qr[:, 2].reshape(C, C).T * WS).astype(F8_NP)
    br = np.asarray(qkv_b, np.float32).reshape(H, 3, CH)
    bq = np.ascontiguousarray(br[:, 0].reshape(C))
    bk = np.ascontiguousarray(br[:, 1].reshape(C))
    bv = np.ascontiguousarray(br[:, 2].reshape(C))
    pw_f = np.asarray(proj_w, np.float32)
    # residual folded in: [pw | I] so proj psum accumulates pw@a + x
    pw2 = np.ascontiguousarray(
        np.concatenate([pw_f.T, np.eye(C, dtype=np.float32)], axis=0)).astype(BF_NP)
    # v bias folded through proj: h = pw @ (a + bv) + pb = pw @ a + (pw@bv + pb)
    pb2 = np.asarray(proj_b, np.float32) + pw_f @ bv
    g8 = np.zeros((128, 8), np.float32)
    g8[np.arange(128), np.arange(128) // 16] = 1.0
    gt8 = np.ascontiguousarray(g8.T)
    g8s = g8 * np.float32(1.0 / GN_N)   # fold the 1/N of the group mean into g8
    cpack = np.zeros((128, 28), np.float32)
    # XS folded into the groupnorm affine (xn8 = 2*xn)
    cpack[:, 0:4] = (np.asarray(norm_scale, np.float32) * XS).reshape(NJ, 128).T
    cpack[:, 4:8] = (np.asarray(norm_bias, np.float32) * XS).reshape(NJ, 128).T
    cpack[:, 8:12] = bq.reshape(NJ, 128).T
    cpack[:, 12:16] = bk.reshape(NJ, 128).T
    cpack[:, 16:20] = pb2.reshape(NJ, 128).T
    cpack[:, 20:28] = g8s
    shared = dict(wq=wq, wk=wk, wv=wv, pw=pw2,
                  cpack=np.ascontiguousarray(cpack),
                  gt8=gt8)
    in_maps = []
    for i in range(B):
        m = dict(shared)
        m["xbf"] = np.ascontiguousarray(x[i].reshape(C, T).astype(BF_NP))
        in_maps.append(m)
    return in_maps


def kernel(x, norm_scale, norm_bias, qkv_w, qkv_b, proj_w, proj_b):
    x = np.asarray(x, dtype=np.float32)
    B, Cc, Hh, Ww = x.shape
    nc = get_nc()
    in_maps = make_in_maps(x, norm_scale, norm_bias, qkv_w, qkv_b, proj_w, proj_b)
    res = run_bass_kernel_spmd(nc, in_maps, core_ids=list(range(B)))
    out = np.stack([res.results[i]["out"] for i in range(B)])
    return out.reshape(B, Cc, Hh, Ww).astype(np.float32)


# revision 15
# speedup vs baseline: 1.2378x; 1.0371x over previous
"""AttentionBlock (GroupNorm32 + QKV 8-head attention + proj + residual) on 8 TRN2 NeuronCores.

Sharding: pure data-parallel over batch B=8 - one batch element per core.

v2 design (fp8 + restructured schedule). Per-core engine walls: ACT must run
64 softmax Exps of 1024 elems (~73us busy - the only engine with exp); the PE
matmul stream is cut below that wall with fp8 DoubleRow (contraction 256/pass)
for the qkv projections, v, and AV (q/k/QK and the x-residual stay bf16 for
accuracy; numerically validated ~5.7e-3 total vs 2e-2 budget):

  - front: input DMAs issued on BOTH hardware DGE queues (Sync + ACT);
    GN stats split DVE (4 sums + 2 sumsq via tensor_tensor_reduce) / ACT
    (2 squares); memsets (ones block, exp bias) on the idle GpSimd; a
    warm-matmul burst holds the HAM clock gate; first Exp at ~16us vs 28.
  - attention: blocks ordered n-OUTER ((p,0) for all p, then (p,1)) so all
    nh=0 attention outputs exist by block 3; proj out-tiles for nh=0 are
    fully contracted + evacuated + DMA'd DURING blocks 4-6 - half the output
    ships mid-attention. exp writes fp8 ew into a persistent 8-slot ring
    (one tensor, so DoubleRow AV can take a [128,2,512] sm-pair AP).
  - AV: stationary [v | ones] fp8 per (sm,h); DoubleRow contracts an
    sm-PAIR per matmul; psum rows 64:128 hold softmax row-sums for free.
  - fillers: kq/v units are naturally micro in fp8 (2 DoubleRow matmuls);
    pumped between sm's (with per-block deadlines) so neither the QK->exp
    chain nor the AV v-dependency ever stalls.
  - tail: only the nh=1 half remains: a3 evac, m2/m3 runs in the freed qk
    psum banks, finals, evacs, DMA issues split across Sync/ACT queues.
"""

import numpy as np
import ml_dtypes
from contextlib import ExitStack

import concourse.bass as bass
import concourse.tile as tile
from concourse import bacc, mybir
from concourse.bass_utils import run_bass_kernel_spmd

F32 = mybir.dt.float32
BF = mybir.dt.bfloat16
F8 = mybir.dt.float8e4
MULT = mybir.AluOpType.mult
ADD = mybir.AluOpType.add
AFT = mybir.ActivationFunctionType
AXX = mybir.AxisListType.X
DR = mybir.MatmulPerfMode.DoubleRow

C, T, H, CH = 512, 1024, 8, 64
NJ = C // 128          # 4 c-tiles
NTM = T // 128         # 8 t-tiles
NPJ = 8                # proj contraction chunks: [pw | I]
EXP_SCALE = float(CH) ** -0.5  # folded (q*s)*(k*s) scale, s = ch**-0.25
EXP_OFF = 4.0                  # exp(l - 4): keeps fp8 ew in e4m3 range
GN_N = 16 * T          # elements per group
WS = 8.0               # host weight scale (w8 = fp8(w*8))
XS = 2.0               # xn scale (xn8 = fp8(2*xn), folded into gn affine)
KQ_DS = 1.0 / (WS * XS)  # psum -> true k/q/v

BF_NP = ml_dtypes.bfloat16
F8_NP = ml_dtypes.float8_e4m3


def build_graph(enable_asserts: bool = False, use_act_dma: bool = True,
                use_gp_memset: bool = True, use_dr: bool = True):
    nc = bacc.Bacc(
        "TRN2",
        target_bir_lowering=False,
        debug=False,
        enable_asserts=enable_asserts,
    )
    act_q = nc.scalar if use_act_dma else nc.sync
    gp_q = nc.gpsimd if use_gp_memset else nc.vector

    def dr_mm(out, lhsT3, rhs3, start, stop, force_plain=False):
        # one fp8 DoubleRow matmul over a k-tile pair, or two plain matmuls
        if use_dr and not force_plain:
            nc.tensor.matmul(out, lhsT3, rhs3, start=start, stop=stop,
                             perf_mode=DR)
        else:
            nc.tensor.matmul(out, lhsT3[:, 0, :], rhs3[:, 0, :],
                             start=start, stop=False)
            nc.tensor.matmul(out, lhsT3[:, 1, :], rhs3[:, 1, :],
                             start=False, stop=stop)
    xbf_d = nc.dram_tensor("xbf", [C, T], BF, kind="ExternalInput").ap()
    wq_d = nc.dram_tensor("wq", [C, C], F8, kind="ExternalInput").ap()
    wk_d = nc.dram_tensor("wk", [C, C], F8, kind="ExternalInput").ap()
    wv_d = nc.dram_tensor("wv", [C, C], F8, kind="ExternalInput").ap()
    pw_d = nc.dram_tensor("pw", [2 * C, C], BF, kind="ExternalInput").ap()
    cp_d = nc.dram_tensor("cpack", [128, 28], F32, kind="ExternalInput").ap()
    gt8_d = nc.dram_tensor("gt8", [8, 128], F32, kind="ExternalInput").ap()
    out_d = nc.dram_tensor("out", [C, T], F32, kind="ExternalOutput").ap()

    with tile.TileContext(nc) as tc, ExitStack() as ctx:
        consts = ctx.enter_context(tc.tile_pool(name="consts", bufs=1))
        bigs = ctx.enter_context(tc.tile_pool(name="bigs", bufs=1))
        work = ctx.enter_context(tc.tile_pool(name="work", bufs=4))
        rinvp = ctx.enter_context(tc.tile_pool(name="rinvp", bufs=4))
        outp = ctx.enter_context(tc.tile_pool(name="outp", bufs=4))
        qk_ps = ctx.enter_context(tc.tile_pool(name="qk_ps", bufs=2, space="PSUM"))
        kv_ps = ctx.enter_context(tc.tile_pool(name="kv_ps", bufs=2, space="PSUM"))
        av_ps = ctx.enter_context(tc.tile_pool(name="av_ps", bufs=2, space="PSUM"))

        # ---- persistent sbuf tensors ----
        xbf = bigs.tile([128, NJ, T], BF)       # bf16 x (GN stats + residual)
        xn8 = bigs.tile([128, NJ, T], F8)       # 2*groupnormed x (fp8)
        q_sb = bigs.tile([128, NJ, T], BF)      # q rows (head-major)
        k_sb = bigs.tile([128, NJ, T], BF)      # k rows (head-major)
        vT2 = bigs.tile([128, NTM, H, 128], F8)  # [v | ones] per (tm, head)
        ew_ring = bigs.tile([128, NTM, 2, 512], F8)  # exp ring: (sm%8, u, t)
        a_sb = bigs.tile([128, NJ, T], BF)      # normalized attention output

        # ---- DMAs split across both HWDGE queues (Sync + ACT) ----
        nc.sync.dma_start(xbf[:, 0, :], xbf_d[0:128, :])
        nc.sync.dma_start(xbf[:, 2, :], xbf_d[256:384, :])
        cpk = consts.tile([128, 28], F32)
        nc.sync.dma_start(cpk[:], cp_d[:])
        gt8_sb = consts.tile([8, 128], F32)
        nc.sync.dma_start(gt8_sb[:], gt8_d[:])

        def stacked(dram, ntiles, width):
            return bass.AP(tensor=dram.tensor, offset=0,
                           ap=[[width, 128], [128 * width, ntiles], [1, width]])
        wk_sb = consts.tile([128, NJ, C], F8)
        wq_sb = consts.tile([128, NJ, C], F8)
        wv_sb = consts.tile([128, NJ, C], F8)
        pw_sb = consts.tile([128, NPJ, C], BF)
        nc.sync.dma_start(wk_sb[:], stacked(wk_d, NJ, C))
        nc.sync.dma_start(wq_sb[:], stacked(wq_d, NJ, C))
        nc.sync.dma_start(pw_sb[:], stacked(pw_d, NPJ, C))

        gns_sb, gnb_sb = cpk[:, 0:4], cpk[:, 4:8]
        bq_sb, bk_sb, pb_sb = cpk[:, 8:12], cpk[:, 12:16], cpk[:, 16:20]
        g8_sb = cpk[:, 20:28]

        # ---- GpSimd: memsets off the critical DVE/ACT paths ----
        zero_sb = consts.tile([128, 1], F32)
        gp_q.memset(zero_sb[:], 0.0)
        moff_sb = consts.tile([128, 1], F32)
        gp_q.memset(moff_sb[:], -EXP_OFF)
        gp_q.memset(vT2[:, :, :, CH:128], 1.0)   # ones block for row-sums

        # DVE: warm tile
        warm_sb = consts.tile([128, 512], BF)
        nc.vector.memset(warm_sb[:], 0.125)

        # ---- ACT queue: xbf j1/j3 DMAs, exp-table preload ----
        act_q.dma_start(xbf[:, 1, :], xbf_d[128:256, :])
        act_q.dma_start(xbf[:, 3, :], xbf_d[384:512, :])
        dume = consts.tile([128, 1], F32)
        nc.scalar.activation(dume[:], zero_sb[:], AFT.Exp, bias=zero_sb[:], scale=1.0)

        # ---- PE warmup: release the HAM clock gate before real matmuls ----
        def warm_mm(n=1):
            for _ in range(n):
                wps = kv_ps.tile([128, 512], F32, tag="kv", name="wps")
                nc.tensor.matmul(wps[:], warm_sb[:, 0:128], warm_sb[:],
                                 start=True, stop=True)

        def warm_dep(rhs):
            wps = kv_ps.tile([128, 512], F32, tag="kv", name="wpsd")
            nc.tensor.matmul(wps[:, 0:rhs.shape[-1]], warm_sb[:, 0:128], rhs,
                             start=True, stop=True)
        warm_mm(10)

        # ---- GroupNorm stats ----
        # ACT: sum(x^2) j0/j2 (Square+accum); DVE: sum(x) j0..j3 and
        # sum(x^2) j1/j3 via tensor_tensor_reduce.
        stats_sb = consts.tile([128, 8], F32)   # sum(x) j=0..3 | sum(x^2) j=0..3
        sq0 = work.tile([128, T], BF, tag="sqs", name="sq0")
        nc.scalar.activation(sq0[:], xbf[:, 0, :], AFT.Square,
                             accum_out=stats_sb[:, 4:5])
        sq2 = work.tile([128, T], BF, tag="sqs", name="sq2")
        nc.scalar.activation(sq2[:], xbf[:, 2, :], AFT.Square,
                             accum_out=stats_sb[:, 6:7])
        act_q.dma_start(wv_sb[:], stacked(wv_d, NJ, C))
        for j in (0, 2, 1, 3):
            nc.vector.tensor_reduce(stats_sb[:, j:j + 1], xbf[:, j, :], AXX, ADD)
            warm_dep(warm_sb[:])
        for j in (1, 3):
            sq = work.tile([128, T], BF, tag="sqs", name="sqv")
            nc.vector.tensor_mul(sq[:], xbf[:, j, :], xbf[:, j, :])
            nc.vector.tensor_reduce(stats_sb[:, 4 + j:5 + j], sq[:], AXX, ADD)
            warm_dep(sq[0:128, 0:512])

        # ---- batched group-reduce + Newton rsqrt + affine ----
        ps_st = kv_ps.tile([128, 512], F32, tag="kv")
        nc.tensor.matmul(ps_st[0:8, 0:8], g8_sb, stats_sb[:], start=True, stop=True)
        stg = work.tile([8, 16], F32, tag="stg")     # mean(0:4)|var(4:8)|t1|t2
        bcin = work.tile([8, 8], F32, tag="bcin")    # mean | rinv
        # g8 host values are pre-scaled by 1/GN_N, so ps_st already holds means
        nc.vector.tensor_copy(stg[:, 0:8], ps_st[0:8, 0:8])
        nc.vector.tensor_mul(stg[:, 8:12], stg[:, 0:4], stg[:, 0:4])
        nc.vector.scalar_tensor_tensor(stg[:, 4:8], stg[:, 8:12], -1.0, stg[:, 4:8],
                                       op0=MULT, op1=ADD)   # var
        nc.vector.tensor_scalar(bcin[:, 4:8], stg[:, 4:8], -0.5, 1.5, op0=MULT, op1=ADD)
        warm_mm(1)
        # Newton: y = y*(1.5 - 0.5*var*y^2)
        nc.vector.tensor_mul(stg[:, 8:12], stg[:, 4:8], bcin[:, 4:8])
        nc.vector.tensor_mul(stg[:, 12:16], stg[:, 8:12], bcin[:, 4:8])
        nc.vector.tensor_scalar(stg[:, 12:16], stg[:, 12:16], -0.5, 1.5, op0=MULT, op1=ADD)
        nc.vector.tensor_mul(bcin[:, 4:8], bcin[:, 4:8], stg[:, 12:16])
        nc.vector.tensor_copy(bcin[:, 0:4], stg[:, 0:4])
        nc.vector.tensor_copy(warm_sb[0:1, 0:1], bcin[0:1, 0:1])
        warm_mm(3)
        ps_pp = kv_ps.tile([128, 512], F32, tag="kv")
        nc.tensor.matmul(ps_pp[0:128, 0:8], gt8_sb[:], bcin[:], start=True, stop=True)
        # dep-injected warm burst keeps the clock gate open through the chain
        nc.vector.tensor_copy(warm_sb[0:1, 0:1], bcin[0:1, 0:1])
        warm_mm(5)
        ab = consts.tile([128, 2, NJ], F32)   # scale | shift (x2 folded via host)
        nc.vector.tensor_mul(ab[:, 0, :], ps_pp[0:128, 4:8], gns_sb)
        t1b = work.tile([128, 4], F32, tag="t1b")
        nc.vector.tensor_mul(t1b[:], ps_pp[0:128, 0:4], ab[:, 0, :])
        nc.vector.tensor_sub(ab[:, 1, :], gnb_sb, t1b[:])

        # ---- xn8 (fp8) interleaved with k0 DoubleRow matmuls ----
        psk0 = kv_ps.tile([128, 512], F32, tag="kv", name="psk0")
        psk1 = kv_ps.tile([128, 512], F32, tag="kv", name="psk1")
        for j in range(NJ):
            nc.vector.tensor_scalar(xn8[:, j, :], xbf[:, j, :],
                                    ab[:, 0, j:j + 1], ab[:, 1, j:j + 1],
                                    op0=MULT, op1=ADD)
            if j % 2 == 1:
                jp = j - 1
                for n in range(2):
                    dr_mm([psk0, psk1][n][:],
                          wk_sb[:, jp:jp + 2, 0:128],
                          xn8[:, jp:jp + 2, 512 * n:512 * (n + 1)],
                          start=(jp == 0), stop=(jp == 2))
        for n in range(2):
            nc.vector.tensor_scalar(k_sb[:, 0, 512 * n:512 * (n + 1)], [psk0, psk1][n][:],
                                    KQ_DS, bk_sb[:, 0:1], op0=MULT, op1=ADD)

        # ---- filler units: 2 DoubleRow matmuls + 1 evac each ----
        def kq_unit(w_sb, b_sb, dst, m, n):
            def emit():
                ps = kv_ps.tile([128, 512], F32, tag="kv", name="ps_kq")
                for jp in (0, 2):
                    dr_mm(ps[:],
                          w_sb[:, jp:jp + 2, 128 * m:128 * (m + 1)],
                          xn8[:, jp:jp + 2, 512 * n:512 * (n + 1)],
                          start=(jp == 0), stop=(jp == 2))
                nc.vector.tensor_scalar(dst[:, m, 512 * n:512 * (n + 1)], ps[:],
                                        KQ_DS, b_sb[:, m:m + 1], op0=MULT, op1=ADD)
            return emit

        def v_unit(tm):
            def emit():
                ps = kv_ps.tile([128, 512], F32, tag="kv", name="ps_v")
                for jp in (0, 2):
                    dr_mm(ps[:],
                          xn8[:, jp:jp + 2, 128 * tm:128 * (tm + 1)],
                          wv_sb[:, jp:jp + 2, :],
                          start=(jp == 0), stop=(jp == 2))
                nc.vector.tensor_scalar(
                    vT2[:, tm, :, 0:CH],
                    ps[:].rearrange("p (h c) -> p h c", c=CH),
                    KQ_DS, None, op0=MULT)
            return emit

        # proj helpers: psum accumulates pw@a + x via [pw | I] chunks.
        # identity chunk for out-tile m is jj=4+m (block-diagonal).
        def proj_chunks(ps, m, nh, chunks, last, first="head"):
            first = chunks[0] if first == "head" else first
            for jj in chunks:
                rhs = (xbf if jj >= NJ else a_sb)
                nc.tensor.matmul(ps,
                                 pw_sb[:, jj, 128 * m:128 * (m + 1)],
                                 rhs[:, jj % NJ, 512 * nh:512 * (nh + 1)],
                                 start=(jj == first), stop=(jj == last))

        osb_ct = [0]

        def proj_evac_dma(ps, m, nh, on_act=False):
            osb = outp.tile([128, 512], F32, tag="osb", name=f"osb{osb_ct[0]}")
            osb_ct[0] += 1
            if on_act:
                nc.scalar.activation(osb[:], ps, AFT.Identity,
                                     bias=pb_sb[:, m:m + 1], scale=1.0)
            else:
                nc.vector.tensor_scalar(osb[:], ps, pb_sb[:, m:m + 1], None, op0=ADD)
            eng = act_q if on_act else nc.sync
            eng.dma_start(out_d[128 * m:128 * (m + 1), 512 * nh:512 * (nh + 1)], osb[:])

        psn0 = [None] * 4
        grp_open = [False]   # no other kv-tag tile may allocate while a
                             # proj accumulation group is open in a kv buf

        def proj_n0_a(m):
            def emit():
                psn0[m] = kv_ps.tile([128, 512], F32, tag="kv", name=f"ps_pj{m}")
                proj_chunks(psn0[m][:], m, 0, [4 + m, 0, 1], -1)
                grp_open[0] = True
            return emit

        def proj_n0_b(m):
            def emit():
                proj_chunks(psn0[m][:], m, 0, [2, 3], 3, first=None)
                proj_evac_dma(psn0[m][:], m, 0)
                grp_open[0] = False
            return emit

        # fillers: (deadline_block, emit). v units feed block-0 AV drains
        # in-stream; k/q for pair p due before block p (n=0) / 4+p (n=1).
        fillers = [
            (1, v_unit(0)), (1, v_unit(1)), (1, v_unit(2)), (1, v_unit(3)),
            (1, kq_unit(wk_sb, bk_sb, k_sb, 1, 0)),
            (1, kq_unit(wk_sb, bk_sb, k_sb, 1, 1)),
            (1, v_unit(4)), (1, v_unit(5)),
            (1, kq_unit(wq_sb, bq_sb, q_sb, 1, 0)),
            (1, v_unit(6)), (1, v_unit(7)),
            (2, kq_unit(wk_sb, bk_sb, k_sb, 2, 0)),
            (2, kq_unit(wk_sb, bk_sb, k_sb, 2, 1)),
            (2, kq_unit(wq_sb, bq_sb, q_sb, 2, 0)),
            (3, kq_unit(wk_sb, bk_sb, k_sb, 3, 0)),
            (3, kq_unit(wk_sb, bk_sb, k_sb, 3, 1)),
            (3, kq_unit(wq_sb, bq_sb, q_sb, 3, 0)),
            (4, kq_unit(wq_sb, bq_sb, q_sb, 0, 1)),
            (5, kq_unit(wq_sb, bq_sb, q_sb, 1, 1)),
            (6, kq_unit(wq_sb, bq_sb, q_sb, 2, 1)),
            (7, kq_unit(wq_sb, bq_sb, q_sb, 3, 1)),
        ]
        # nh=0 proj out-tiles: fully computable + shippable from block 4 on
        late = [(36, proj_n0_a(0)), (38, proj_n0_b(0)),
                (42, proj_n0_a(1)), (44, proj_n0_b(1)),
                (48, proj_n0_a(2)), (50, proj_n0_b(2)),
                (54, proj_n0_a(3)), (56, proj_n0_b(3))]
        # nh=1 m0/m1 pre-chunks (x + a0..a2) held in the kv banks to the tail
        psn1 = [None, None]

        def n1_pre(m):
            def emit():
                psn1[m] = kv_ps.tile([128, 512], F32, tag="kv", name=f"psn1_{m}")
                proj_chunks(psn1[m][:], m, 1, [4 + m, 0, 1, 2], -1)
            return emit


        def pump(slot, big=False):
            if fillers:
                fillers.pop(0)[1]()
            elif big and late and slot >= late[0][0]:
                late.pop(0)[1]()
            elif slot < 62 and not grp_open[0]:
                warm_mm(1)

        # ---- q0 n=0 ----
        kq_unit(wq_sb, bq_sb, q_sb, 0, 0)()

        # ---- attention: n-OUTER blocks; AV = fp8 DoubleRow over sm-pairs ----
        def emit_qk(p, n, sm):
            psw = qk_ps.tile([128, T], F32, tag="qk", name="psw")
            nc.tensor.matmul(psw[:, 0:512],
                             k_sb[0:64, p, 128 * sm:128 * (sm + 1)],
                             q_sb[0:64, p, 512 * n:512 * (n + 1)],
                             start=True, stop=True, tile_position=(0, 0))
            nc.tensor.matmul(psw[:, 512:1024],
                             k_sb[64:128, p, 128 * sm:128 * (sm + 1)],
                             q_sb[64:128, p, 512 * n:512 * (n + 1)],
                             start=True, stop=True, tile_position=(64, 0))
            nc.scalar.activation(ew_ring[:, sm, :, :],
                                 psw[:].rearrange("p (u t) -> p u t", u=2),
                                 AFT.Exp, bias=moff_sb[:], scale=EXP_SCALE)

        def emit_evac(p, n, psa):
            # custom-DVE ops need partition-0 based SBUF APs on HW
            for u in range(2):
                rs = rinvp.tile([64, 512], F32, tag="rs", name="rs")
                nc.vector.tensor_copy(rs[:], psa[u][64:128, :])
                rinv = rinvp.tile([64, 512], F32, tag="rinv", name="rinv")
                nc.vector.reciprocal_approx_fast(rinv[:], rs[:])
                nc.vector.tensor_mul(a_sb[64 * u:64 * (u + 1), p, 512 * n:512 * (n + 1)],
                                     psa[u][0:CH, :], rinv[:])

        blocks = [(p, n) for n in range(2) for p in range(NJ)]
        bpsa = {}
        pend = []

        def drain(auto_evac=True):
            bi, p, n, sp = pend.pop(0)
            # late blocks run AV as plain matmuls: 2x the instructions at
            # the same math keeps PE density above the HAM clock-gate
            # threshold once the fillers have run dry
            for u in range(2):
                dr_mm(bpsa[bi][u][:],
                      vT2[:, 2 * sp:2 * sp + 2, 2 * p + u, :],
                      ew_ring[:, 2 * sp:2 * sp + 2, u, :],
                      start=(sp == 0), stop=(sp == 3), force_plain=(bi >= 4))
            if sp == 3 and auto_evac:
                emit_evac(p, n, bpsa[bi])
                return True
            return False

        for bi, (p, n) in enumerate(blocks):
            while fillers and fillers[0][0] <= bi:
                fillers.pop(0)[1]()
            psa0 = av_ps.tile([128, 512], F32, tag="av", name="psa0")
            psa1 = av_ps.tile([128, 512], F32, tag="av", name="psa1")
            bpsa[bi] = [psa0, psa1]
            for sp in range(4):
                slot = 8 * bi + 2 * sp
                emit_qk(p, n, 2 * sp)
                pump(slot)
                emit_qk(p, n, 2 * sp + 1)
                pend.append((bi, p, n, sp))
                evd = False
                if len(pend) > 2:
                    evd = drain()
                if not evd:
                    pump(slot + 1, big=True)

        # ---- tail: nh=1 only ----
        drain()                      # AV(last, sp=2)
        lbi, lp, ln_, lsp = pend[0]
        drain(auto_evac=False)       # AV(last, sp=3)
        n1_pre(0)()
        n1_pre(1)()
        # m2/m3 nh=1 pre-chunks in the qk psum banks freed by the last exps
        psq2 = qk_ps.tile([128, T], F32, tag="qk", name="psq2")
        psq3 = qk_ps.tile([128, T], F32, tag="qk", name="psq3")
        proj_chunks(psq2[:, 0:512], 2, 1, [6, 0, 1, 2], -1)
        proj_chunks(psq3[:, 0:512], 3, 1, [7, 0, 1, 2], -1)
        emit_evac(lp, ln_, bpsa[lbi])
        finals = [psn1[0][:], psn1[1][:], psq2[:, 0:512], psq3[:, 0:512]]
        for m in range(4):
            nc.tensor.matmul(finals[m],
                             pw_sb[:, 3, 128 * m:128 * (m + 1)],
                             a_sb[:, 3, 512:1024],
                             start=False, stop=True)
        # evacuations + DMA issues alternate Sync / ACT queues
        for m in range(4):
            proj_evac_dma(finals[m], m, 1, on_act=(m % 2 == 1))

    nc.compile()
    return nc


_NC_CACHE = {}


def get_nc():
    if "nc" not in _NC_CACHE:
        _NC_CACHE["nc"] = build_graph()
    return _NC_CACHE["nc"]


def make_in_maps(x, norm_scale, norm_bias, qkv_w, qkv_b, proj_w, proj_b):
    x = np.asarray(x, dtype=np.float32)
    B = x.shape[0]
    qr = np.asarray(qkv_w, np.float32).reshape(H, 3, CH, C)
    wq = np.ascontiguousarray(qr[:, 0].reshape(C, C).T * WS).astype(F8_NP)
    wk = np.ascontiguousarray(qr[:, 1].reshape(C, C).T * WS).astype(F8_NP)
    wv = np.ascontiguousarray(qr[:, 2].reshape(C, C).T * WS).astype(F8_NP)
    br = np.asarray(qkv_b, np.float32).reshape(H, 3, CH)
    bq = np.ascontiguousarray(br[:, 0].reshape(C))
    bk = np.ascontiguousarray(br[:, 1].reshape(C))
    bv = np.ascontiguousarray(br[:, 2].reshape(C))
    pw_f = np.asarray(proj_w, np.float32)
    # residual folded in: [pw | I] so proj psum accumulates pw@a + x
    pw2 = np.ascontiguousarray(
        np.concatenate([pw_f.T, np.eye(C, dtype=np.float32)], axis=0)).astype(BF_NP)
    # v bias folded through proj: h = pw @ (a + bv) + pb = pw @ a + (pw@bv + pb)
    pb2 = np.asarray(proj_b, np.float32) + pw_f @ bv
    g8 = np.zeros((128, 8), np.float32)
    g8[np.arange(128), np.arange(128) // 16] = 1.0
    gt8 = np.ascontiguousarray(g8.T)
    g8s = g8 * np.float32(1.0 / GN_N)   # fold the 1/N of the group mean into g8
    cpack = np.zeros((128, 28), np.float32)
    # XS folded into the groupnorm affine (xn8 = 2*xn)
    cpack[:, 0:4] = (np.asarray(norm_scale, np.float32) * XS).reshape(NJ, 128).T
    cpack[:, 4:8] = (np.asarray(norm_bias, np.float32) * XS).reshape(NJ, 128).T
    cpack[:, 8:12] = bq.reshape(NJ, 128).T
    cpack[:, 12:16] = bk.reshape(NJ, 128).T
    cpack[:, 16:20] = pb2.reshape(NJ, 128).T
    cpack[:, 20:28] = g8s
    shared = dict(wq=wq, wk=wk, wv=wv, pw=pw2,
                  cpack=np.ascontiguousarray(cpack),
                  gt8=gt8)
    in_maps = []
    for i in range(B):
        m = dict(shared)
        m["xbf"] = np.ascontiguousarray(x[i].reshape(C, T).astype(BF_NP))
        in_maps.append(m)
    return in_maps


def kernel(x, norm_scale, norm_bias, qkv_w, qkv_b, proj_w, proj_b):
    x = np.asarray(x, dtype=np.float32)
    B, Cc, Hh, Ww = x.shape
    nc = get_nc()
    in_maps = make_in_maps(x, norm_scale, norm_bias, qkv_w, qkv_b, proj_w, proj_b)
    res = run_bass_kernel_spmd(nc, in_maps, core_ids=list(range(B)))
    out = np.stack([res.results[i]["out"] for i in range(B)])
    return out.reshape(B, Cc, Hh, Ww).astype(np.float32)
